# revision 14
# baseline (speedup 1.0000x reference)
"""Trainium2 Bass kernel for nn_CustomPoisson: bit-exact reproduction of
jax.random.poisson (threefry2x32 partitionable, Knuth algorithm) + spike
encoding, sharded over 8 NeuronCores along the pixel axis.

Self-contained: kernel(img) -> bool [500, 262144].

v2: 6 Knuth draws (counts capped at 6; ~1.4k of 131M elements mis-capped,
well under the 2e-2 rel-err budget), product-domain check (prod > exp(-lam)
uploaded per pixel) replacing the on-device log replica, and u16 limb
storage so the rotate/xor half of threefry runs in the DVE's 2x/4x modes.
Adds still go through u32 temps (u16 writeback saturates on overflow).
"""
import sys

for _p in ("/opt/trn_rl_repo",):
    if _p not in sys.path:
        sys.path.append(_p)

import numpy as np
from contextlib import ExitStack

import jax as _jax

# Persistent compilation cache: without it every run_bass_kernel_spmd call
# re-runs client-side BIR verify/optimize + DVE table generation (~0.5s).
_jax.config.update("jax_compilation_cache_dir", "/tmp/jaxcomp_cache")
_jax.config.update("jax_persistent_cache_min_entry_size_bytes", -1)
_jax.config.update("jax_persistent_cache_min_compile_time_secs", 0)

from concourse import bass, mybir
from concourse import tile
from concourse.bass_utils import run_bass_kernel_spmd

ALU = mybir.AluOpType
U32 = np.uint32
F32 = np.float32
M16 = 0xFFFF

N_PIX = 262144
T = 500
TP = 512                                  # padded per-pixel spike stride
N_CORES = 8
PIX_PER_CORE = N_PIX // N_CORES          # 32768
PL = 4                                    # pixels per partition per tile
FREE = PL * T                             # 2000
FREEP = PL * TP                           # 2048
NBYTES = PL * (TP // 8)                   # 256 packed bytes per partition/tile
N_TILES = PIX_PER_CORE // (128 * PL)      # 64
# Uneven chunk split: the axon output download is serial at ~35MB/s and a
# chunk's download can only start after its whole program finishes, while
# chunk executions serialize on the cores. Geometrically increasing sizes
# (ratio ~ download_rate/exec_rate ~ 1.8) equalize every partial
# exec-prefix + download-suffix sum, minimizing the pipeline makespan.
CHUNK_SPLIT = [24, 40]
J_DRAWS = 6                               # Knuth draws; counts capped at 6

ROUNDS = [13, 15, 26, 6, 17, 29, 16, 24, 13, 15, 26, 6, 17, 29, 16, 24, 13, 15, 26, 6]


def _np_threefry2x32(k1, k2, x0, x1):
    k1 = U32(k1); k2 = U32(k2)
    ks2 = U32(k1 ^ k2 ^ U32(0x1BD11BDA))
    x0 = (np.asarray(x0, U32) + k1).astype(U32)
    x1 = (np.asarray(x1, U32) + k2).astype(U32)
    ks = [k1, k2, ks2]
    for i in range(5):
        for r in ROUNDS[4 * i:4 * i + 4]:
            x0 = (x0 + x1).astype(U32)
            x1 = ((x1 << U32(r)) | (x1 >> U32(32 - r))).astype(U32)
            x1 = x1 ^ x0
        x0 = (x0 + ks[(i + 1) % 3]).astype(U32)
        x1 = (x1 + ks[(i + 2) % 3] + U32(i + 1)).astype(U32)
    return x0, x1


def _subkeys(n):
    """Subkeys consumed by the Knuth while-loop for jax.random.key(42)."""
    rng = (U32(0), U32(42))
    out = []
    c = np.arange(2, dtype=U32)
    for _ in range(n):
        o0, o1 = _np_threefry2x32(rng[0], rng[1], np.zeros(2, U32), c)
        rng = (int(o0[0]), int(o1[0]))
        out.append((int(o0[1]), int(o1[1])))
    return out


SUBKEYS = _subkeys(J_DRAWS)

# --- emit helpers ------------------------------------------------------------

U32DT = mybir.dt.uint32
U16DT = mybir.dt.uint16
F32DT = mybir.dt.float32
U8DT = mybir.dt.uint8

_BITVEC_OPS = {ALU.bitwise_and, ALU.bitwise_or, ALU.bitwise_xor, ALU.bitwise_not,
               ALU.logical_shift_left, ALU.logical_shift_right,
               ALU.arith_shift_left, ALU.arith_shift_right}


def _imm_for(val, op, dt):
    if op in _BITVEC_OPS:
        return mybir.ImmediateValue(dtype=dt, value=int(val))
    return mybir.ImmediateValue(dtype=F32DT, value=float(val))


def _ts(eng, out, in0, s1, op0, s2=None, op1=None, dt=U16DT):
    """tensor_scalar (single tensor + immediates / scalar-AP): 2x/4x modes."""
    ins = [eng.lower_ap(in0),
           _imm_for(s1, op0, dt) if not isinstance(s1, bass.AP) else eng.lower_ap(s1)]
    kw = dict(op0=op0)
    if op1 is not None:
        ins.append(_imm_for(s2, op1, dt) if not isinstance(s2, bass.AP) else eng.lower_ap(s2))
        kw["op1"] = op1
    return eng.add_instruction(mybir.InstTensorScalarPtr(
        name=eng.bass.get_next_instruction_name(), ins=ins, outs=[eng.lower_ap(out)], **kw))


def _stt(eng, out, in0, scalar, in1, op0, op1, dt=U16DT):
    """scalar_tensor_tensor: out = (in0 op0 scalar) op1 in1."""
    sc = eng.lower_ap(scalar) if isinstance(scalar, bass.AP) else _imm_for(scalar, op0, dt)
    return eng.add_instruction(mybir.InstTensorScalarPtr(
        name=eng.bass.get_next_instruction_name(), is_scalar_tensor_tensor=True,
        op0=op0, op1=op1, ins=[eng.lower_ap(in0), sc, eng.lower_ap(in1)],
        outs=[eng.lower_ap(out)]))


def legalize_waits(nc, max_waits=1):
    """Walrus accepts one sync wait per instruction; move extras to NOPs."""
    engs = {}
    for attr in ("vector", "scalar", "gpsimd", "sync", "tensor"):
        e = getattr(nc, attr, None)
        if e is not None and hasattr(e, "engine"):
            engs[e.engine] = e

    def make_nop(etype):
        ins = engs[etype].nop()
        for bb in nc.main_func.blocks:
            for k in range(len(bb.instructions) - 1, -1, -1):
                if bb.instructions[k] is ins.ins:
                    del bb.instructions[k]
                    return ins.ins
        return ins.ins

    n = 0
    for bb in nc.main_func.blocks:
        out = []
        for ins in bb.instructions:
            si = ins.sync_info
            if si is not None and si.on_wait is not None and len(si.on_wait) > max_waits:
                waits = list(si.on_wait)
                extra, keep = waits[:-max_waits], waits[-max_waits:]
                for w in extra:
                    nop = make_nop(ins.engine)
                    nop.sync_info = mybir.SyncInfo(on_wait=[w], on_update=[])
                    out.append(nop)
                ins.sync_info = mybir.SyncInfo(on_wait=keep, on_update=list(si.on_update or []))
                n += 1
            out.append(ins)
        bb.instructions[:] = out
    return n


def build_nc(n_tiles, j_draws=J_DRAWS):
    nc = bass.Bass()
    v = nc.vector
    P, F = 128, FREE

    elam_in = nc.declare_dram_parameter("elam", [P, n_tiles * PL], F32DT, isOutput=False)
    cb_in = nc.declare_dram_parameter("cbase", [P, 2], F32DT, isOutput=False)
    spk_out = nc.declare_dram_parameter("spikes", [P, n_tiles * NBYTES], U8DT, isOutput=True)

    with tile.TileContext(nc) as tc, ExitStack() as ctx:
        pp = ctx.enter_context(tc.tile_pool(name="persist", bufs=1))
        cntl = pp.tile([P, F], U32DT, name="cntl")   # 16-bit limb values
        cnth = pp.tile([P, F], U32DT, name="cnth")
        tcolsp = pp.tile([P, FREEP], F32DT, name="tcolsp")
        elam_sm = pp.tile([P, PL], F32DT, name="elam_sm")
        cbase = pp.tile([P, 2], F32DT, name="cbase")
        spad = pp.tile([P, FREEP], F32DT, name="spad")
        nc.sync.dma_start(cbase[:], cb_in[:])

        wp = ctx.enter_context(tc.tile_pool(name="work", bufs=1))
        x0l = wp.tile([P, F], U32DT, name="x0l")
        x0h = wp.tile([P, F], U32DT, name="x0h")
        x1l = wp.tile([P, F], U32DT, name="x1l")
        x1h = wp.tile([P, F], U32DT, name="x1h")
        t1 = wp.tile([P, F], U32DT, name="t1")
        t2 = wp.tile([P, F], U32DT, name="t2")
        elam = wp.tile([P, F], F32DT, name="elam")
        prod = wp.tile([P, F], F32DT, name="prod")
        kc = wp.tile([P, F], F32DT, name="kc")
        df = wp.tile([P, F], F32DT, name="df")
        vF = wp.tile([P, F], F32DT, name="vF")
        spkf = wp.tile([P, FREEP], F32DT, name="spkf")
        pacc = wp.tile([P, NBYTES], F32DT, name="pacc")
        pby = wp.tile([P, NBYTES], U8DT, name="pby")

        # on-device counter: cnt = p*FREE + f + chunk/core base, kept as two
        # u32 tiles with 16-bit limb values. iota values <= 256000, exact.
        nc.gpsimd.iota(t1[:], [[1, F]], base=0, channel_multiplier=F)
        _ts(v, t2[:], t1[:], M16, ALU.bitwise_and, dt=U32DT)
        _ts(v, t2[:], t2[:], cbase[:, 1:2], ALU.add, dt=U32DT)           # + base_lo
        _ts(v, cntl[:], t2[:], M16, ALU.bitwise_and, dt=U32DT)
        _ts(v, t2[:], t2[:], 16, ALU.logical_shift_right, dt=U32DT)     # carry
        _ts(v, t1[:], t1[:], 16, ALU.logical_shift_right, dt=U32DT)
        _ts(v, t1[:], t1[:], cbase[:, 0:1], ALU.add, dt=U32DT)          # + base_hi
        v.tensor_tensor(cnth[:], t1[:], t2[:], ALU.add)
        # time columns t in [0,TP) per PL-segment; [s*TP, s*TP+T) slices give
        # the [0,T) columns too.
        nc.gpsimd.iota(tcolsp[:], [[0, PL], [1, TP]], base=0, channel_multiplier=0,
                       allow_small_or_imprecise_dtypes=True)
        v.memset(spad[:], -1.0)

        def draw(j):
            """threefry2x32(SUBKEYS[j-1], (0, cnt)) -> vF = u = (o0^o1)>>9 * 2^-23."""
            k1, k2 = SUBKEYS[j - 1]
            ks = [k1, k2, k1 ^ k2 ^ 0x1BD11BDA]
            C01 = (ks[0] + ks[1]) % (1 << 32)
            # x1 = cnt + ks1   (canonical)
            _ts(v, t1[:], cntl[:], ks[1] & M16, ALU.add, dt=U32DT)
            _ts(v, x1l[:], t1[:], M16, ALU.bitwise_and, dt=U32DT)
            _ts(v, t2[:], t1[:], 16, ALU.logical_shift_right, dt=U32DT)
            _stt(v, x1h[:], cnth[:], (ks[1] >> 16) & M16, t2[:], ALU.add, ALU.add, dt=U32DT)
            _ts(v, x1h[:], x1h[:], M16, ALU.bitwise_and, dt=U32DT)
            # round 1: x0 = cnt + C01  (hi may stay junky)
            _ts(v, t1[:], cntl[:], C01 & M16, ALU.add, dt=U32DT)
            _ts(v, x0l[:], t1[:], M16, ALU.bitwise_and, dt=U32DT)
            _ts(v, t2[:], t1[:], 16, ALU.logical_shift_right, dt=U32DT)
            _stt(v, x0h[:], cnth[:], (C01 >> 16) & M16, t2[:], ALU.add, ALU.add, dt=U32DT)

            def rotxor(r):
                # x1 = rotl32(x1, r) ^ x0 ; x1 canonical afterwards
                if r == 16:
                    v.tensor_tensor(t1[:], x1l[:], x0h[:], ALU.bitwise_xor)
                    _ts(v, t1[:], t1[:], M16, ALU.bitwise_and, dt=U32DT)
                    _stt(v, x1l[:], x1h[:], M16, x0l[:], ALU.bitwise_and, ALU.bitwise_xor, dt=U32DT)
                    v.tensor_copy(x1h[:], t1[:])
                    return
                if r < 16:
                    _ts(v, t1[:], x1l[:], 16 - r, ALU.logical_shift_right, dt=U32DT)
                    _stt(v, t1[:], x1h[:], r, t1[:], ALU.logical_shift_left, ALU.bitwise_or, dt=U32DT)
                    _ts(v, t2[:], x1h[:], 16 - r, ALU.logical_shift_right, dt=U32DT)
                    _stt(v, t2[:], x1l[:], r, t2[:], ALU.logical_shift_left, ALU.bitwise_or, dt=U32DT)
                else:
                    rp = r - 16
                    _ts(v, t1[:], x1h[:], 32 - r, ALU.logical_shift_right, dt=U32DT)
                    _stt(v, t1[:], x1l[:], rp, t1[:], ALU.logical_shift_left, ALU.bitwise_or, dt=U32DT)
                    _ts(v, t2[:], x1l[:], 32 - r, ALU.logical_shift_right, dt=U32DT)
                    _stt(v, t2[:], x1h[:], rp, t2[:], ALU.logical_shift_left, ALU.bitwise_or, dt=U32DT)
                v.tensor_tensor(t1[:], t1[:], x0h[:], ALU.bitwise_xor)
                _ts(v, x1h[:], t1[:], M16, ALU.bitwise_and, dt=U32DT)
                _stt(v, x1l[:], t2[:], M16, x0l[:], ALU.bitwise_and, ALU.bitwise_xor, dt=U32DT)

            rotxor(ROUNDS[0])
            for ridx in range(1, 20):
                kal = kah = 0
                if ridx % 4 == 0:
                    g = ridx // 4  # 1..4
                    ka = ks[g % 3]
                    kb = (ks[(g + 1) % 3] + g) % (1 << 32)
                    kal, kah = ka & M16, (ka >> 16) & M16
                    # x1 += kb (canonical)
                    _ts(v, t1[:], x1l[:], kb & M16, ALU.add, dt=U32DT)
                    _ts(v, x1l[:], t1[:], M16, ALU.bitwise_and, dt=U32DT)
                    _ts(v, t2[:], t1[:], 16, ALU.logical_shift_right, dt=U32DT)
                    _stt(v, x1h[:], x1h[:], (kb >> 16) & M16, t2[:], ALU.add, ALU.add, dt=U32DT)
                    _ts(v, x1h[:], x1h[:], M16, ALU.bitwise_and, dt=U32DT)
                # x0 += x1 (+ka)
                if kal or kah:
                    _stt(v, t1[:], x0l[:], kal, x1l[:], ALU.add, ALU.add, dt=U32DT)
                    _stt(v, x0h[:], x0h[:], kah, x1h[:], ALU.add, ALU.add, dt=U32DT)
                else:
                    v.tensor_tensor(t1[:], x0l[:], x1l[:], ALU.add)
                    v.tensor_tensor(x0h[:], x0h[:], x1h[:], ALU.add)
                _ts(v, x0l[:], t1[:], M16, ALU.bitwise_and, dt=U32DT)
                _ts(v, t2[:], t1[:], 16, ALU.logical_shift_right, dt=U32DT)
                v.tensor_tensor(x0h[:], x0h[:], t2[:], ALU.add)
                rotxor(ROUNDS[ridx])
            # final injection (g=5): o0 = x0 + ks[2], o1 = x1 + ks[0]+5
            ka = ks[2]
            kb = (ks[0] + 5) % (1 << 32)
            _ts(v, t1[:], x0l[:], ka & M16, ALU.add, dt=U32DT)
            _ts(v, x0l[:], t1[:], M16, ALU.bitwise_and, dt=U32DT)
            _ts(v, t2[:], t1[:], 16, ALU.logical_shift_right, dt=U32DT)
            _stt(v, x0h[:], x0h[:], (ka >> 16) & M16, t2[:], ALU.add, ALU.add, dt=U32DT)
            _ts(v, t1[:], x1l[:], kb & M16, ALU.add, dt=U32DT)
            _ts(v, x1l[:], t1[:], M16, ALU.bitwise_and, dt=U32DT)
            _ts(v, t2[:], t1[:], 16, ALU.logical_shift_right, dt=U32DT)
            _stt(v, x1h[:], x1h[:], (kb >> 16) & M16, t2[:], ALU.add, ALU.add, dt=U32DT)
            # u = (((o0h^o1h) << 7 | (o0l^o1l) >> 9) & 0x7FFFFF) * 2^-23
            v.tensor_tensor(t2[:], x0h[:], x1h[:], ALU.bitwise_xor)
            v.tensor_tensor(t1[:], x0l[:], x1l[:], ALU.bitwise_xor)
            _ts(v, t1[:], t1[:], 9, ALU.logical_shift_right, dt=U32DT)
            _stt(v, t1[:], t2[:], 7, t1[:], ALU.logical_shift_left, ALU.bitwise_or, dt=U32DT)
            _ts(v, t1[:], t1[:], 0x7FFFFF, ALU.bitwise_and, dt=U32DT)
            _ts(v, vF[:], t1[:], float(F32(2.0 ** -23)), ALU.mult, dt=F32DT)

        with tc.For_i(0, n_tiles, 1, hint_engines=(mybir.EngineType.DVE,)) as i:
            # broadcast per-pixel exp(-lam) to the [P, PL*T] working layout
            nc.sync.dma_start(elam_sm[:], elam_in[:, bass.ds(i * PL, PL)])
            v.memset(elam[:], 0.0)
            for pl in range(PL):
                _ts(v, elam[:, pl * T:(pl + 1) * T], elam[:, pl * T:(pl + 1) * T],
                    elam_sm[:, pl:pl + 1], ALU.add, dt=F32DT)
            # check 1: kc = (elam < 1) i.e. lam > 0 ; prod = 1
            _ts(v, kc[:], elam[:], 1.0, ALU.is_lt, dt=F32DT)
            v.memset(prod[:], 1.0)

            for j in range(1, j_draws + 1):
                draw(j)
                v.tensor_tensor(prod[:], prod[:], vF[:], ALU.mult)
                # check j+1: kc += (prod > elam)
                v.tensor_tensor(df[:], prod[:], elam[:], ALU.subtract)
                _stt(v, kc[:], df[:], 0.0, kc[:], ALU.is_gt, ALU.add, dt=F32DT)

            # ---- epilogue: counts -> spikes; bit-pack --------------------
            _ts(v, kc[:], kc[:], -1.0, ALU.add, 0.0, ALU.max, dt=F32DT)  # counts
            for s in range(PL):
                v.tensor_tensor(df[:, s * T:(s + 1) * T], kc[:, s * T:(s + 1) * T],
                                tcolsp[:, s * TP:s * TP + T], ALU.add)   # ends
                v.tensor_tensor_scan(spad[:, s * TP:s * TP + T],
                                     df[:, s * T:(s + 1) * T],
                                     df[:, s * T:(s + 1) * T], -1.0, ALU.max, ALU.bypass)
            v.tensor_tensor(spkf[:], spad[:], tcolsp[:], ALU.is_gt)
            sbits = spkf[:].rearrange("p (y b) -> p y b", b=8)
            _ts(v, pacc[:], sbits[:, :, 0], 1.0, ALU.mult, dt=F32DT)
            for b in range(1, 8):
                _stt(v, pacc[:], sbits[:, :, b], float(1 << b), pacc[:],
                     ALU.mult, ALU.add, dt=F32DT)
            v.tensor_copy(pby[:], pacc[:])
            nc.sync.dma_start(spk_out[:, bass.ds(i * NBYTES, NBYTES)], pby[:])

            # advance counters by one tile stride (128*PL*T flat indices)
            STRIDE = 128 * PL * T
            _ts(v, t1[:], cntl[:], STRIDE & M16, ALU.add, dt=U32DT)
            _ts(v, cntl[:], t1[:], M16, ALU.bitwise_and, dt=U32DT)
            _ts(v, t2[:], t1[:], 16, ALU.logical_shift_right, dt=U32DT)
            _stt(v, cnth[:], cnth[:], (STRIDE >> 16) & M16, t2[:], ALU.add, ALU.add, dt=U32DT)

    legalize_waits(nc)
    return nc


# --- host side ---------------------------------------------------------------

def _host_inputs(img):
    """Input maps per chunk per core. Pixel mapping within core c:
    p_glob = c*PIX_PER_CORE + i*(128*PL) + p*PL + pl, with chunk k owning
    tiles [tile_base, tile_base + nt)."""
    img = np.asarray(img, F32)
    elam = np.exp(-img.astype(np.float64)).astype(F32)
    chunk_maps = []
    tile_base = 0
    for nt in CHUNK_SPLIT:
        in_maps = []
        for c in range(N_CORES):
            base = c * PIX_PER_CORE + tile_base * 128 * PL
            npx = nt * 128 * PL
            elam_sm = np.ascontiguousarray(
                elam[base:base + npx].reshape(nt, 128, PL)
                .transpose(1, 0, 2).reshape(128, nt * PL))
            cb0 = c * PIX_PER_CORE * T + tile_base * 128 * PL * T
            cb = np.empty((128, 2), F32)
            cb[:, 0] = cb0 >> 16
            cb[:, 1] = cb0 & M16
            in_maps.append({"elam": elam_sm, "cbase": cb})
        chunk_maps.append(in_maps)
        tile_base += nt
    return chunk_maps


STAGGER_S = 0.01


def _run_chunks(ncs, chunk_maps):
    """One spmd call per chunk from a small thread pool so chunk k's
    transfers overlap chunk k+1's execution."""
    import time as _time
    from concurrent.futures import ThreadPoolExecutor
    if len(chunk_maps) == 1:
        return [run_bass_kernel_spmd(ncs[0], chunk_maps[0],
                                     core_ids=list(range(N_CORES))).results]

    def _worker(k):
        _time.sleep(k * STAGGER_S)
        return run_bass_kernel_spmd(ncs[k], chunk_maps[k],
                                    core_ids=list(range(N_CORES)))

    with ThreadPoolExecutor(len(chunk_maps)) as ex:
        futs = [ex.submit(_worker, k) for k in range(len(chunk_maps))]
        return [f.result().results for f in futs]


def _assemble(chunk_results):
    """chunk_results[k][c]["spikes"] [128, nt*NBYTES] u8 (bit-packed along T,
    8 steps/byte little-endian, 64 bytes per pixel) -> out [T, N] bool.

    Unpacks along the (contiguous) byte axis into a pixel-major [N, 512]
    master buffer, then returns the zero-copy transposed view of the first
    T columns -- no 131MB strided transpose on the host.
    """
    master = np.empty((N_PIX, TP), np.uint8)
    for c in range(N_CORES):
        tile_base = 0
        for k, nt in enumerate(CHUNK_SPLIT):
            spk = chunk_results[k][c]["spikes"].reshape(128, nt, PL, TP // 8)
            pkt = np.ascontiguousarray(spk.transpose(1, 0, 2, 3))  # [nt,128,PL,64]
            bits = np.unpackbits(pkt, axis=-1, bitorder="little")  # [nt,128,PL,512]
            base = c * PIX_PER_CORE + tile_base * 128 * PL
            npx = nt * 128 * PL
            master[base:base + npx] = bits.reshape(npx, TP)
            tile_base += nt
    return master[:, :T].view(bool).T


_NC_CACHE = {}


def _get_ncs():
    ncs = []
    for nt in CHUNK_SPLIT:
        if nt not in _NC_CACHE:
            _NC_CACHE[nt] = build_nc(nt)
        ncs.append(_NC_CACHE[nt])
    return ncs


def kernel(img):
    img = np.asarray(img)
    assert img.shape == (N_PIX,)
    ncs = _get_ncs()
    chunk_maps = _host_inputs(img)
    res = _run_chunks(ncs, chunk_maps)
    return _assemble(res)
